# revision 22
# baseline (speedup 1.0000x reference)
"""Trainium2 Bass kernel for nn_BasicTransformerBlock (self-attn + cross-attn + GeGLU FFN).

Sharding: 8 cores; core c handles batch b = c//2, query-token half = c%2.
The host rolls each core's copy of the batch sequence so its own 1024 query
tokens are always rows 0:1024 (self-attention sums over all keys, so the
roll is free). K/V are computed redundantly per core; no collectives.

v3 (latency-hiding rewrite):
  - Attention is ACT(exp)-bound: ~1.2us of exp per 128-key step vs ~0.9us
    of PE work. All projection matmuls for the NEXT head pair (and the next
    phase's K/V prep) are emitted as "filler" chunks interleaved into the
    attention loop, so the in-order PE queue eats them during exp waits
    instead of idling.
  - attn@V runs one key-tile behind the scores (software pipeline), so no
    PE instruction waits on an exp issued in the same step.
  - V-projection chunks are interleaved into the LayerNorm+transpose loop.
  - LayerNorm uses single-pass bn_stats/bn_aggr + reciprocal_approx_fast.
  - Softmax denominators: copy from PSUM, reciprocal_approx_fast, gpsimd
    partition broadcast; head B's context is moved to partitions 64:128 via
    a shifted-identity matmul so ctx^T is stored per head-PAIR [128, TC]
    and the O projections contract K=128.
  - dtypes: bf16 matmuls everywhere (incl. FF1 weights); residual stream,
    LN and softmax statistics in fp32.
"""
import numpy as np
import ml_dtypes
from contextlib import ExitStack

import concourse.bass as bass
import concourse.tile as tile
from concourse import bacc, mybir, bass_utils, library_config
from concourse.masks import make_identity

F32 = mybir.dt.float32
BF16 = mybir.dt.bfloat16
AF = mybir.ActivationFunctionType
ALU = mybir.AluOpType
AX = mybir.AxisListType

B, N, D = 4, 2048, 1024
S, CD = 77, 768
H, DH = 16, 64
FF = 4096
NCORES = 8
TC = N // 2
NT = TC // 128
NTB = N // 128
KD = D // 128
KC = CD // 128
NP = H // 2
EPS = 1e-5

_CACHE = {}


def build_program():
    nc = bacc.Bacc("TRN2", target_bir_lowering=False, debug=False)

    def din(name, shape, dt=F32):
        return nc.dram_tensor(name, list(shape), dt, kind="ExternalInput").ap()

    xb = din("xb", (N, D))
    ctxp = din("ctxp", (128, CD))
    wq1 = din("wq1", (D, D), BF16); bq1 = din("bq1", (D,))
    wk1 = din("wk1", (D, D), BF16); bk1 = din("bk1", (D,))
    wv1 = din("wv1", (D, D), BF16); bv1 = din("bv1", (D,))
    wo1 = din("wo1", (D, D), BF16); bo1 = din("bo1", (D,))
    wq2 = din("wq2", (D, D), BF16); bq2 = din("bq2", (D,))
    wk2 = din("wk2", (CD, D), BF16)
    wv2 = din("wv2", (CD, D), BF16)
    wo2 = din("wo2", (D, D), BF16); bo2 = din("bo2", (D,))
    vones = din("vones", (128, H), BF16)
    wp = din("wp", (D, 2 * FF), BF16); bp = din("bp", (2 * FF,))
    wf = din("wf", (FF, D), BF16); bf = din("bf", (D,))
    y = nc.dram_tensor("y", [TC, D], F32, kind="ExternalOutput").ap()

    with tile.TileContext(nc) as tc, ExitStack() as ctx:
        nc.gpsimd.load_library(library_config.attn)
        small = ctx.enter_context(tc.tile_pool(name="small", bufs=4))
        singles = ctx.enter_context(tc.tile_pool(name="singles", bufs=1))
        xpool = ctx.enter_context(tc.tile_pool(name="xpool", bufs=2))
        evpool = ctx.enter_context(tc.tile_pool(name="evpool", bufs=2))
        xres = ctx.enter_context(tc.tile_pool(name="xres", bufs=1))
        keep = ctx.enter_context(tc.tile_pool(name="keep", bufs=1))

        identf = singles.tile([128, 128], F32, tag="identf", name="identf")
        make_identity(nc, identf[:])
        identb = singles.tile([128, 128], BF16, tag="identb", name="identb")
        nc.vector.tensor_copy(identb[:], identf[:])
        eps_t = singles.tile([128, 1], F32, tag="eps_t", name="eps_t")
        nc.vector.memset(eps_t[:], EPS)
        # shifted identity: shiftT.T @ x places x (partitions 0:64) into
        # partitions 64:128 of the output
        shiftT = singles.tile([64, 128], BF16, tag="shiftT", name="shiftT")
        nc.vector.memset(shiftT[:], 0.0)
        nc.vector.tensor_copy(shiftT[0:64, 64:128], identb[0:64, 0:64])

        # ---------------- helpers ----------------
        def ln_tile(x_t, xn_t):
            """LayerNorm (no affine): xn = (x - mean) * rstd, one [128, D] tile."""
            xr = x_t.rearrange("p (s f) -> p s f", s=2)
            st = small.tile([128, 2, 6], F32, tag="ln_st", name="ln_st")
            nc.vector.bn_stats(st[:, 0, :], xr[:, 0, :])
            nc.vector.bn_stats(st[:, 1, :], xr[:, 1, :])
            mv = small.tile([128, 2], F32, tag="ln_mv", name="ln_mv")
            nc.vector.bn_aggr(mv[:], st[:])
            std = small.tile([128, 1], F32, tag="ln_std", name="ln_std")
            nc.scalar.activation(std[:], mv[:, 1:2], AF.Sqrt, bias=eps_t[:])
            rstd = small.tile([128, 1], F32, tag="ln_rstd", name="ln_rstd")
            nc.vector.reciprocal_approx_fast(rstd[:], std[:])
            nc.vector.tensor_scalar(xn_t, x_t, mv[:, 0:1], rstd[:],
                                    op0=ALU.subtract, op1=ALU.mult)

        def transpose_to(psum_pool, src, dsts, ident):
            for k, dst in enumerate(dsts):
                ps = psum_pool.tile([128, 128], src.dtype, tag="mm", name="tr")
                nc.tensor.transpose(ps[:], src[:, k * 128:(k + 1) * 128], ident)
                nc.any.tensor_copy(dst, ps[:])

        def make_xT(pool, psum_pool, x_src_tile_fn, ntiles, tagp,
                    fillers_at=None, out_list=None):
            """LN + transpose -> feature-major chunks [128, ntiles*128] bf16.
            fillers_at: {t: [closure, ...]} run after tile t's transposes."""
            xT = [pool.tile([128, ntiles * 128], BF16, tag=f"{tagp}_{k}",
                            name=f"{tagp}_{k}") for k in range(KD)]
            if out_list is not None:
                out_list.extend(xT)
            for t in range(ntiles):
                x_t = x_src_tile_fn(t)
                xn = xpool.tile([128, D], BF16, tag="xn", name="xn")
                ln_tile(x_t, xn[:])
                transpose_to(psum_pool, xn[:],
                             [xT[k][:, t * 128:(t + 1) * 128] for k in range(KD)],
                             identb[:])
                if fillers_at and t in fillers_at:
                    for f in fillers_at[t]:
                        f()
            return xT

        def bias_fm(bap, nslices, name):
            t = singles.tile([128, nslices], F32, tag=f"bfm_{name}", name=f"bfm_{name}")
            nc.sync.dma_start(t[:], bap.rearrange("(a p) -> p a", p=128))
            return t

        def bias_bc(pool, bap, width, name, dt=F32):
            t = pool.tile([128, width], dt, tag=f"bbc_{name}", name=f"bbc_{name}")
            eng = nc.sync if dt == F32 else nc.gpsimd
            eng.dma_start(t[:], bass.AP(tensor=bap.tensor, offset=bap.offset,
                                        ap=[[0, 128]] + bap.ap))
            return t

        def load_colblock(pool, w_ap, col0, ncols, tag, nk, bufs=2):
            t = pool.tile([128, nk, ncols], BF16, tag=tag, name=tag, bufs=bufs)
            src = w_ap[:, col0:col0 + ncols].rearrange("(a p) n -> p a n", p=128)
            nc.sync.dma_start(t[:], src)
            return t

        bq1_fm = bias_fm(bq1, KD, "bq1")
        bk1_fm = bias_fm(bk1, KD, "bk1")
        bq2_fm = bias_fm(bq2, KD, "bq2")
        bp_fm = bias_fm(bp, 2 * FF // 128, "bp")

        x1 = [xres.tile([128, D], F32, tag=f"x1_{t}", name=f"x1_{t}")
              for t in range(NT)]
        ctxT1 = [keep.tile([128, TC], BF16, tag=f"ctx1_{p}", name=f"ctx1_{p}")
                 for p in range(NP)]

        def xb_src(t):
            xt = xpool.tile([128, D], F32, tag="xt", name="xt")
            nc.sync.dma_start(xt[:], xb[t * 128:(t + 1) * 128, :])
            return xt[:]

        with tc.tile_pool(name="mm_ps", bufs=2, space="PSUM") as mm_ps, \
             tc.tile_pool(name="sc_ps", bufs=2, space="PSUM") as sc_ps, \
             tc.tile_pool(name="av_ps", bufs=1, space="PSUM") as av_ps, \
             tc.tile_pool(name="apool", bufs=2) as apool:

            def attn_pair(kTp, qTp, vslice_fn, ctx_dst, n_t, filler=(),
                          av_from_sc=False):
                """Both heads of a pair: scores -> exp -> (attn@V | denom),
                attn@V two key-tiles behind; `filler` closures are spread
                evenly into the PE-idle slots of the ACT-bound loop."""
                filler = list(filler)
                nsteps = (TC // 512) * (n_t + 2)
                due = [(len(filler) * (s + 1)) // nsteps for s in range(nsteps)]
                ndone = [0]
                step = [0]

                def fill():
                    want = due[min(step[0], nsteps - 1)]
                    while ndone[0] < want and ndone[0] < len(filler):
                        filler[ndone[0]]()
                        ndone[0] += 1
                    step[0] += 1

                LAG = 2

                def av_mm(avA, avB, pend_item, last):
                    pt, pexA, pexB = pend_item
                    nc.tensor.matmul(avA[0:65, :], vslice_fn(pt, 0), pexA[:],
                                     start=(pt == 0), stop=last)
                    nc.tensor.matmul(avB[0:65, :], vslice_fn(pt, 1), pexB[:],
                                     start=(pt == 0), stop=last)

                for j in range(TC // 512):
                    jsl = slice(j * 512, (j + 1) * 512)
                    if av_from_sc:
                        avA = sc_ps.tile([128, 512], F32, tag="scA", name="avA")
                        avB = sc_ps.tile([128, 512], F32, tag="scB", name="avB")
                    else:
                        avA = av_ps.tile([128, 512], F32, tag="avA", name="avA",
                                         bufs=1)
                        avB = av_ps.tile([128, 512], F32, tag="avB", name="avB",
                                         bufs=1)
                    pend = []
                    for t in range(n_t):
                        tsl = slice(t * 128, (t + 1) * 128)
                        scA = sc_ps.tile([128, 512], F32, tag="scA", name="scA")
                        scB = sc_ps.tile([128, 512], F32, tag="scB", name="scB")
                        nc.tensor.matmul(scA[:], kTp[0:64, tsl], qTp[0:64, jsl],
                                         start=True, stop=True)
                        nc.tensor.matmul(scB[:], kTp[64:128, tsl],
                                         qTp[64:128, jsl], start=True, stop=True)
                        exA = apool.tile([128, 512], BF16, tag="exA", name="exA",
                                         bufs=3)
                        exB = apool.tile([128, 512], BF16, tag="exB", name="exB",
                                         bufs=3)
                        nc.scalar.activation(exA[:], scA[:], AF.Exp)
                        nc.scalar.activation(exB[:], scB[:], AF.Exp)
                        pend.append((t, exA, exB))
                        if len(pend) > LAG:
                            av_mm(avA, avB, pend.pop(0), False)
                        fill()
                    while pend:
                        item = pend.pop(0)
                        av_mm(avA, avB, item, not pend)
                        fill()
                    dnA = small.tile([1, 512], F32, tag="dnA", name="dnA", bufs=1)
                    dnB = small.tile([1, 512], F32, tag="dnB", name="dnB", bufs=1)
                    nc.vector.tensor_copy(dnA[:], avA[64:65, :])
                    nc.vector.tensor_copy(dnB[:], avB[64:65, :])
                    rA = small.tile([1, 512], F32, tag="rA", name="rA", bufs=1)
                    rB = small.tile([1, 512], F32, tag="rB", name="rB", bufs=1)
                    nc.vector.reciprocal_approx_fast(rA[:], dnA[:])
                    nc.vector.reciprocal_approx_fast(rB[:], dnB[:])
                    rbA = apool.tile([64, 512], F32, tag="rbA", name="rbA", bufs=1)
                    rbB = apool.tile([64, 512], F32, tag="rbB", name="rbB", bufs=1)
                    nc.gpsimd.partition_broadcast(rbA[:], rA[:])
                    nc.gpsimd.partition_broadcast(rbB[:], rB[:])
                    fill()
                    nc.vector.tensor_mul(ctx_dst[0:64, jsl], avA[0:64, :], rbA[:])
                    cB = apool.tile([64, 512], BF16, tag="cB", name="cB", bufs=1)
                    nc.vector.tensor_mul(cB[:], avB[0:64, :], rbB[:])
                    ps2 = mm_ps.tile([128, 512], F32, tag="mm", name="shiftB")
                    nc.tensor.matmul(ps2[:], shiftT[:], cB[:], start=True,
                                     stop=True)
                    nc.any.tensor_copy(ctx_dst[64:128, jsl], ps2[64:128, :])
                    fill()
                while ndone[0] < len(filler):
                    filler[ndone[0]]()
                    ndone[0] += 1

            # =================================================================
            # Phase 1: self-attention
            # =================================================================
            p1stack = ExitStack()
            p1xnT = p1stack.enter_context(tc.tile_pool(name="p1xnT", bufs=1))
            p1w = p1stack.enter_context(tc.tile_pool(name="p1w", bufs=1))
            bv1_bc = bias_bc(p1xnT, bv1, D, "bv1", dt=BF16)

            xn1T = []  # filled by make_xT below; filler closures capture the list
            pair_state = [dict() for _ in range(NP)]
            vh_state = [dict() for _ in range(2)]

            def v_start(half):
                def go():
                    vh_state[half]['wvh'] = load_colblock(
                        p1w, wv1, half * 512, 512, "wvh", KD, bufs=1)
                    vh_state[half]['vsth'] = p1w.tile(
                        [128, NTB * 8 * 65], BF16, tag="vsth", name="vsth", bufs=2)
                return go

            def v_chunk(half, t):
                def go():
                    tsl = slice(t * 128, (t + 1) * 128)
                    ps = mm_ps.tile([128, 512], F32, tag="mm", name="vproj")
                    for k in range(KD):
                        nc.tensor.matmul(ps[:], xn1T[k][:, tsl],
                                         vh_state[half]['wvh'][:, k, :],
                                         start=(k == 0), stop=(k == KD - 1))
                    vt = vh_state[half]['vsth'][:, t * 520:(t + 1) * 520].rearrange(
                        "p (h c) -> p h c", h=8)
                    nc.vector.tensor_add(
                        vt[:, :, 0:64], ps[:].rearrange("p (h c) -> p h c", h=8),
                        bv1_bc[:, half * 512:half * 512 + 512].rearrange(
                            "p (h c) -> p h c", h=8))
                    nc.vector.memset(vt[:, :, 64:65], 1.0)
                return go

            def p_start(m):
                def go():
                    pair_state[m]['wkm'] = load_colblock(p1w, wk1, m * 128, 128,
                                                         "wkm", KD)
                    pair_state[m]['wqm'] = load_colblock(p1w, wq1, m * 128, 128,
                                                         "wqm", KD)
                    pair_state[m]['kT'] = p1w.tile([128, N], BF16, tag="kT",
                                                   name="kT", bufs=2)
                    pair_state[m]['qT'] = p1w.tile([128, TC], BF16, tag="qT",
                                                   name="qT", bufs=2)
                return go

            def k_chunk(m, jb):
                def go():
                    jbs = slice(jb * 512, (jb + 1) * 512)
                    ps = mm_ps.tile([128, 512], F32, tag="mm", name="kproj")
                    for k in range(KD):
                        nc.tensor.matmul(ps[:], pair_state[m]['wkm'][:, k, :],
                                         xn1T[k][:, jbs], start=(k == 0),
                                         stop=(k == KD - 1))
                    nc.vector.tensor_scalar(pair_state[m]['kT'][:, jbs], ps[:],
                                            bk1_fm[:, m:m + 1], None, op0=ALU.add)
                return go

            def q_chunk(m, jb):
                def go():
                    jbs = slice(jb * 512, (jb + 1) * 512)
                    ps = mm_ps.tile([128, 512], F32, tag="mm", name="qproj")
                    for k in range(KD):
                        nc.tensor.matmul(ps[:], pair_state[m]['wqm'][:, k, :],
                                         xn1T[k][:, jbs], start=(k == 0),
                                         stop=(k == KD - 1))
                    nc.vector.tensor_scalar(pair_state[m]['qT'][:, jbs], ps[:],
                                            bq1_fm[:, m:m + 1], None, op0=ALU.add)
                return go

            def pair_chunks(m):
                return ([p_start(m)] + [k_chunk(m, jb) for jb in range(4)]
                        + [q_chunk(m, jb) for jb in range(2)])

            # ---- P2 K/V prep chunks (run during pair 7's attention) ----
            ctx_sb = keep.tile([128, CD], F32, tag="ctx_sb", name="ctx_sb")
            cT = [keep.tile([128, 128], BF16, tag=f"cT_{k}", name=f"cT_{k}")
                  for k in range(KC)]
            k2p = [keep.tile([128, 128], BF16, tag=f"k2p_{m}", name=f"k2p_{m}")
                   for m in range(NP)]
            v2st = keep.tile([128, H * 65], BF16, tag="v2st", name="v2st")
            p2k_state = {}

            def p2_prep_chunks():
                out = []

                def c_load():
                    nc.sync.dma_start(ctx_sb[:], ctxp)
                    transpose_to(mm_ps, ctx_sb[:], [cT[k][:] for k in range(KC)],
                                 identf[:])
                out.append(c_load)

                def k2_chunk(m2):
                    def go():
                        wkm2 = load_colblock(p1w, wk2, m2 * 128, 128, "wk2m", KC, bufs=1)
                        ps = mm_ps.tile([128, 512], F32, tag="mm", name="k2proj")
                        for k in range(KC):
                            nc.tensor.matmul(ps[:, 0:128], wkm2[:, k, :],
                                             cT[k][:], start=(k == 0),
                                             stop=(k == KC - 1))
                        nc.any.tensor_copy(k2p[m2][:], ps[:, 0:128])
                    return go
                out.extend(k2_chunk(m2) for m2 in range(NP))

                def v2_chunk(n):
                    def go():
                        wvn = load_colblock(p1w, wv2, n * 512, 512, "wv2n", KC,
                                            bufs=1)
                        ps = mm_ps.tile([128, 512], F32, tag="mm", name="v2proj")
                        for k in range(KC):
                            nc.tensor.matmul(ps[:], cT[k][:], wvn[:, k, :],
                                             start=(k == 0), stop=(k == KC - 1))
                        dst = v2st[:].rearrange("p (h c) -> p h c", h=H)[
                            :, n * 8:(n + 1) * 8, 0:64]
                        nc.vector.tensor_copy(
                            dst, ps[:].rearrange("p (h c) -> p h c", h=8))
                    return go
                out.extend(v2_chunk(n) for n in range(2))

                def ones_load():
                    onescol = v2st[:].rearrange("p (h c) -> p h c", h=H)[:, :, 64:65]
                    nc.sync.dma_start(onescol,
                                      vones.rearrange("p (h o) -> p h o", o=1))
                out.append(ones_load)
                return out

            # ---- make_xT with V-proj + pair-0 fillers ----
            fillers_at = {0: [v_start(0)]}
            for t in range(NTB):
                fillers_at.setdefault(t, []).append(v_chunk(0, t))
            fillers_at[1].append(p_start(0))
            for jb in range(4):
                fillers_at.setdefault(4 * jb + 3, []).append(k_chunk(0, jb))
            for jb in range(2):
                fillers_at.setdefault(8 * jb + 7, []).append(q_chunk(0, jb))
            make_xT(p1xnT, mm_ps, xb_src, NTB, "xn1T", fillers_at,
                    out_list=xn1T)

            # ---- attention over pairs with interleaved fillers ----
            vh1 = [v_start(1)] + [v_chunk(1, t) for t in range(NTB)]
            fill_for = {
                0: pair_chunks(1),
                1: pair_chunks(2) + vh1[0:6],
                2: pair_chunks(3) + vh1[6:12],
                3: vh1[12:17] + pair_chunks(4),
                4: pair_chunks(5),
                5: pair_chunks(6),
                6: pair_chunks(7),
                7: p2_prep_chunks(),
            }
            for m in range(NP):
                half, pi = m // 4, m % 4
                attn_pair(pair_state[m]['kT'][:], pair_state[m]['qT'][:],
                          lambda t, hh, half=half, pi=pi: vh_state[half]['vsth'][
                              :, t * 520 + (pi * 2 + hh) * 65:
                              t * 520 + (pi * 2 + hh + 1) * 65],
                          ctxT1[m][:], NTB, fill_for[m])

            p1stack.close()

            # ---- O1 projection + residual ----
            with tc.tile_pool(name="p1o", bufs=1) as p1o:
                bo1_bc = bias_bc(p1o, bo1, D, "bo1")
                wo1p = [p1o.tile([128, D], BF16, tag=f"wo1_{p}", name=f"wo1_{p}")
                        for p in range(NP)]
                for p in range(NP):
                    nc.sync.dma_start(wo1p[p][:], wo1[p * 128:(p + 1) * 128, :])
                for t in range(NT):
                    tsl = slice(t * 128, (t + 1) * 128)
                    xo = xpool.tile([128, D], F32, tag="xt", name="xt_res")
                    nc.sync.dma_start(xo[:], xb[t * 128:(t + 1) * 128, :])
                    ps0 = mm_ps.tile([128, 512], F32, tag="mm", name="oproj0")
                    ps1 = mm_ps.tile([128, 512], F32, tag="mm", name="oproj1")
                    for p in range(NP):
                        nc.tensor.matmul(ps0[:], ctxT1[p][:, tsl],
                                         wo1p[p][:, 0:512], start=(p == 0),
                                         stop=(p == NP - 1))
                        nc.tensor.matmul(ps1[:], ctxT1[p][:, tsl],
                                         wo1p[p][:, 512:1024], start=(p == 0),
                                         stop=(p == NP - 1))
                    for n, ps in ((0, ps0), (1, ps1)):
                        nsl = slice(n * 512, (n + 1) * 512)
                        tmp = evpool.tile([128, 512], F32, tag="otmp", name="otmp")
                        nc.any.tensor_add(tmp[:], ps[:], bo1_bc[:, nsl])
                        nc.vector.tensor_add(x1[t][:, nsl], tmp[:], xo[:, nsl])

            # =================================================================
            # Phase 2: cross-attention (updates x1 in place)
            # =================================================================
            with tc.tile_pool(name="p2", bufs=1) as p2, \
                 tc.tile_pool(name="p2w", bufs=2) as p2w:
                ctxT2 = [p2.tile([128, TC], BF16, tag=f"ctx2_{p}", name=f"ctx2_{p}")
                         for p in range(NP)]
                xn2T = []
                q2_state = [dict() for _ in range(NP)]

                def q2_start(m):
                    def go():
                        q2_state[m]['wqm'] = load_colblock(p2w, wq2, m * 128, 128,
                                                           f"wq2m_{m}", KD, bufs=1)
                        q2_state[m]['qT'] = p2w.tile([128, TC], BF16,
                                                     tag=f"q2T_{m}",
                                                     name="q2T", bufs=1)
                    return go

                def q2_chunk(m, jb):
                    def go():
                        jbs = slice(jb * 512, (jb + 1) * 512)
                        ps = mm_ps.tile([128, 512], F32, tag="mm", name="q2proj")
                        for k in range(KD):
                            nc.tensor.matmul(ps[:], q2_state[m]['wqm'][:, k, :],
                                             xn2T[k][:, jbs], start=(k == 0),
                                             stop=(k == KD - 1))
                        nc.vector.tensor_scalar(q2_state[m]['qT'][:, jbs], ps[:],
                                                bq2_fm[:, m:m + 1], None,
                                                op0=ALU.add)
                    return go

                f2 = {0: [q2_start(m) for m in range(NP)]}
                for m in range(NP):
                    f2.setdefault(3 + m % 4, []).append(q2_chunk(m, 0))
                make_xT(p2, mm_ps, lambda t: x1[t][:], NT, "xn2T", f2,
                        out_list=xn2T)
                for m in range(NP):
                    q2_chunk(m, 1)()
                for m in range(NP):
                    attn_pair(k2p[m][:], q2_state[m]['qT'][:],
                              lambda t, hh, m=m: v2st[:, (2 * m + hh) * 65:
                                                      (2 * m + hh + 1) * 65],
                              ctxT2[m][:], 1, (), av_from_sc=True)

                with tc.tile_pool(name="p2o", bufs=1) as p2o:
                    bo2_bc = bias_bc(p2o, bo2, D, "bo2")
                    wo2p = [p2o.tile([128, D], BF16, tag=f"wo2_{p}", name=f"wo2_{p}")
                            for p in range(NP)]
                    for p in range(NP):
                        nc.sync.dma_start(wo2p[p][:], wo2[p * 128:(p + 1) * 128, :])
                    for t in range(NT):
                        tsl = slice(t * 128, (t + 1) * 128)
                        ps0 = mm_ps.tile([128, 512], F32, tag="mm", name="o2proj0")
                        ps1 = mm_ps.tile([128, 512], F32, tag="mm", name="o2proj1")
                        for p in range(NP):
                            nc.tensor.matmul(ps0[:], ctxT2[p][:, tsl],
                                             wo2p[p][:, 0:512], start=(p == 0),
                                             stop=(p == NP - 1))
                            nc.tensor.matmul(ps1[:], ctxT2[p][:, tsl],
                                             wo2p[p][:, 512:1024], start=(p == 0),
                                             stop=(p == NP - 1))
                        for n, ps in ((0, ps0), (1, ps1)):
                            nsl = slice(n * 512, (n + 1) * 512)
                            tmp = evpool.tile([128, 512], F32, tag="otmp",
                                              name="o2tmp")
                            nc.any.tensor_add(tmp[:], ps[:], bo2_bc[:, nsl])
                            nc.vector.tensor_add(x1[t][:, nsl], tmp[:],
                                                 x1[t][:, nsl])

        # =================================================================
        # Phase 3: GeGLU FFN (x1 now holds x2); FF2 partials DMA-accumulated
        # =================================================================
        with tc.tile_pool(name="mm3_ps", bufs=4, space="PSUM") as mm3, \
             tc.tile_pool(name="p3", bufs=1) as p3:
            xn3T = make_xT(p3, mm3, lambda t: x1[t][:], NT, "xn3T")
            bf_bc = bias_bc(p3, bf, D, "bf")
            # x1 += bf (safe: xn3T already computed)
            for t in range(NT):
                nc.vector.tensor_add(x1[t][:], x1[t][:], bf_bc[:])

            NG = 2
            MPG = FF // 128 // NG
            with tc.tile_pool(name="p3w", bufs=1) as p3w:
                for G in range(NG):
                    wfn = [p3w.tile([128, D], BF16, tag=f"wf_{i}", name=f"wf_{i}",
                                    bufs=1) for i in range(MPG)]
                    for i in range(MPG):
                        nc.sync.dma_start(wfn[i][:],
                                          wf[(G * MPG + i) * 128:
                                             (G * MPG + i + 1) * 128, :])
                    agT = [p3w.tile([128, TC], BF16, tag=f"agT_{i}",
                                    name=f"agT_{i}", bufs=1) for i in range(MPG)]
                    for i in range(MPG):
                        mc = G * MPG + i
                        wpa = load_colblock(p3w, wp, mc * 128, 128, "wpa", KD)
                        wpg = load_colblock(p3w, wp, FF + mc * 128, 128, "wpg", KD)
                        for jb in range(TC // 512):
                            jsl = slice(jb * 512, (jb + 1) * 512)
                            ps_a = mm3.tile([128, 512], F32, tag="mm", name="ff1a")
                            for k in range(KD):
                                nc.tensor.matmul(ps_a[:], wpa[:, k, :],
                                                 xn3T[k][:, jsl], start=(k == 0),
                                                 stop=(k == KD - 1))
                            ps_g = mm3.tile([128, 512], F32, tag="mm", name="ff1g")
                            for k in range(KD):
                                nc.tensor.matmul(ps_g[:], wpg[:, k, :],
                                                 xn3T[k][:, jsl], start=(k == 0),
                                                 stop=(k == KD - 1))
                            gel = evpool.tile([128, 512], F32, tag="gel",
                                              name="gel")
                            nc.scalar.activation(gel[:], ps_g[:], AF.Gelu,
                                                 bias=bp_fm[:, FF // 128 + mc:
                                                            FF // 128 + mc + 1])
                            nc.vector.scalar_tensor_tensor(agT[i][:, jsl], ps_a[:],
                                                           bp_fm[:, mc:mc + 1],
                                                           gel[:], op0=ALU.add,
                                                           op1=ALU.mult)
                    for t in range(NT):
                        tsl = slice(t * 128, (t + 1) * 128)
                        ps0 = mm3.tile([128, 512], F32, tag="mm", name="ff2a")
                        ps1 = mm3.tile([128, 512], F32, tag="mm", name="ff2b")
                        for i in range(MPG):
                            nc.tensor.matmul(ps0[:], agT[i][:, tsl],
                                             wfn[i][:, 0:512], start=(i == 0),
                                             stop=(i == MPG - 1))
                            nc.tensor.matmul(ps1[:], agT[i][:, tsl],
                                             wfn[i][:, 512:1024], start=(i == 0),
                                             stop=(i == MPG - 1))
                        for n, ps in ((0, ps0), (1, ps1)):
                            nsl = slice(n * 512, (n + 1) * 512)
                            ev = evpool.tile([128, 512], F32, tag="yev", name="yev",
                                             bufs=3)
                            if G == 0:
                                nc.vector.tensor_add(ev[:], ps[:], x1[t][:, nsl])
                                nc.sync.dma_start(y[t * 128:(t + 1) * 128, nsl],
                                                  ev[:])
                            else:
                                nc.any.tensor_copy(ev[:], ps[:])
                                nc.gpsimd.dma_start(y[t * 128:(t + 1) * 128, nsl],
                                                    ev[:], accum_op=ALU.add)

    nc.compile()
    return nc


def _prep_inputs(inputs):
    """Host-side weight transforms + per-core input maps."""
    f = np.float32
    bff = ml_dtypes.bfloat16
    x = np.asarray(inputs["x"], f)
    context = np.asarray(inputs["context"], f)
    g1, b1 = np.asarray(inputs["g1"], f), np.asarray(inputs["b1"], f)
    g2, b2 = np.asarray(inputs["g2"], f), np.asarray(inputs["b2"], f)
    g3, b3 = np.asarray(inputs["g3"], f), np.asarray(inputs["b3"], f)
    sc = f(DH ** -0.5)

    Wq1, Wk1, Wv1 = (np.asarray(inputs[k], f) for k in ("Wq1", "Wk1", "Wv1"))
    Wq2, Wp = np.asarray(inputs["Wq2"], f), np.asarray(inputs["Wp"], f)

    def c(a, dt=None):
        a = np.ascontiguousarray(a)
        return a.astype(dt) if dt is not None else a

    shared = dict(
        wq1=c(g1[:, None] * Wq1 * sc, bff), bq1=c(b1 @ Wq1 * sc),
        wk1=c(g1[:, None] * Wk1, bff), bk1=c(b1 @ Wk1),
        wv1=c(g1[:, None] * Wv1, bff), bv1=c(b1 @ Wv1),
        wo1=c(np.asarray(inputs["Wo1"], f), bff), bo1=c(np.asarray(inputs["bo1"], f)),
        wq2=c(g2[:, None] * Wq2 * sc, bff), bq2=c(b2 @ Wq2 * sc),
        wk2=c(np.asarray(inputs["Wk2"], f), bff),
        wv2=c(np.asarray(inputs["Wv2"], f), bff),
        wo2=c(np.asarray(inputs["Wo2"], f), bff), bo2=c(np.asarray(inputs["bo2"], f)),
        wp=c(g3[:, None] * Wp, bff),
        bp=c(np.asarray(inputs["bp"], f) + b3 @ Wp),
        wf=c(np.asarray(inputs["Wf"], f), bff), bf=c(np.asarray(inputs["bf"], f)),
        vones=np.ascontiguousarray(np.where(np.arange(128)[:, None] < S, 1, 0).repeat(H, 1).astype(bff)),
    )
    ctxpad = np.zeros((B, 128, CD), f)
    ctxpad[:, :S, :] = context

    in_maps = []
    for cid in range(NCORES):
        b, half = cid // 2, cid % 2
        m = dict(shared)
        m["xb"] = np.ascontiguousarray(np.roll(x[b], -half * TC, axis=0))
        m["ctxp"] = np.ascontiguousarray(ctxpad[b])
        in_maps.append(m)
    return in_maps


def run(inputs, trace=False):
    if "nc" not in _CACHE:
        _CACHE["nc"] = build_program()
    nc = _CACHE["nc"]
    in_maps = _prep_inputs(inputs)
    res = bass_utils.run_bass_kernel_spmd(nc, in_maps, core_ids=list(range(NCORES)),
                                          trace=trace)
    out = np.empty((B, N, D), np.float32)
    for cid in range(NCORES):
        b, half = cid // 2, cid % 2
        out[b, half * TC:(half + 1) * TC] = res.results[cid]["y"]
    return out, res


def kernel(**inputs):
    out, _ = run(inputs, trace=False)
    return out


# revision 24
# speedup vs baseline: 1.0196x; 1.0196x over previous
"""Trainium2 Bass kernel for nn_BasicTransformerBlock (self-attn + cross-attn + GeGLU FFN).

Sharding: 8 cores; core c handles batch b = c//2, query-token half = c%2.
The host rolls each core's copy of the batch sequence so its own 1024 query
tokens are always rows 0:1024 (self-attention sums over all keys, so the
roll is free). K/V are computed redundantly per core; no collectives.

v4 (latency-hiding rewrite):
  - Attention is ACT(exp)-bound: ~1.2us of exp per 128-key step vs ~0.9us
    of PE work. All projection matmuls for the NEXT head pair (and the next
    phase's K/V prep) are emitted as "filler" chunks interleaved into the
    attention loop, so the in-order PE queue eats them during exp waits
    instead of idling.
  - attn@V runs two key-tiles behind the scores (software pipeline), so no
    PE instruction waits on an exp issued in the same step; filler chunks
    are spread evenly over the steps to keep the PE streaming (and the HAM
    clock-gate warm).
  - V-projection chunks are interleaved into the LayerNorm+transpose loop.
  - LayerNorm uses single-pass bn_stats/bn_aggr + reciprocal_approx_fast.
  - Softmax denominators: copy from PSUM, reciprocal_approx_fast, gpsimd
    partition broadcast; head B's context is moved to partitions 64:128 via
    a shifted-identity matmul so ctx^T is stored per head-PAIR [128, TC]
    and the O projections contract K=128.
  - dtypes: bf16 matmuls everywhere (incl. FF1 weights); residual stream,
    LN and softmax statistics in fp32.
"""
import numpy as np
import ml_dtypes
from contextlib import ExitStack

import concourse.bass as bass
import concourse.tile as tile
from concourse import bacc, mybir, bass_utils, library_config
from concourse.masks import make_identity

F32 = mybir.dt.float32
BF16 = mybir.dt.bfloat16
AF = mybir.ActivationFunctionType
ALU = mybir.AluOpType
AX = mybir.AxisListType

B, N, D = 4, 2048, 1024
S, CD = 77, 768
H, DH = 16, 64
FF = 4096
NCORES = 8
TC = N // 2
NT = TC // 128
NTB = N // 128
KD = D // 128
KC = CD // 128
NP = H // 2
EPS = 1e-5

_CACHE = {}


def build_program():
    nc = bacc.Bacc("TRN2", target_bir_lowering=False, debug=False)

    def din(name, shape, dt=F32):
        return nc.dram_tensor(name, list(shape), dt, kind="ExternalInput").ap()

    xb = din("xb", (N, D))
    ctxp = din("ctxp", (128, CD))
    wq1 = din("wq1", (D, D), BF16); bq1 = din("bq1", (D,))
    wk1 = din("wk1", (D, D), BF16); bk1 = din("bk1", (D,))
    wv1 = din("wv1", (D, D), BF16); bv1 = din("bv1", (D,))
    wo1 = din("wo1", (D, D), BF16); bo1 = din("bo1", (D,))
    wq2 = din("wq2", (D, D), BF16); bq2 = din("bq2", (D,))
    wk2 = din("wk2", (CD, D), BF16)
    wv2 = din("wv2", (CD, D), BF16)
    wo2 = din("wo2", (D, D), BF16); bo2 = din("bo2", (D,))
    vones = din("vones", (128, H), BF16)
    wp = din("wp", (D, 2 * FF), BF16); bp = din("bp", (2 * FF,))
    wf = din("wf", (FF, D), BF16); bf = din("bf", (D,))
    y = nc.dram_tensor("y", [TC, D], F32, kind="ExternalOutput").ap()

    with tile.TileContext(nc) as tc, ExitStack() as ctx:
        nc.gpsimd.load_library(library_config.attn)
        small = ctx.enter_context(tc.tile_pool(name="small", bufs=4))
        singles = ctx.enter_context(tc.tile_pool(name="singles", bufs=1))
        xpool = ctx.enter_context(tc.tile_pool(name="xpool", bufs=2))
        evpool = ctx.enter_context(tc.tile_pool(name="evpool", bufs=2))
        xres = ctx.enter_context(tc.tile_pool(name="xres", bufs=1))
        keep = ctx.enter_context(tc.tile_pool(name="keep", bufs=1))

        identf = singles.tile([128, 128], F32, tag="identf", name="identf")
        make_identity(nc, identf[:])
        identb = singles.tile([128, 128], BF16, tag="identb", name="identb")
        nc.vector.tensor_copy(identb[:], identf[:])
        eps_t = singles.tile([128, 1], F32, tag="eps_t", name="eps_t")
        nc.vector.memset(eps_t[:], EPS)
        # shifted identity: shiftT.T @ x places x (partitions 0:64) into
        # partitions 64:128 of the output
        shiftT = singles.tile([64, 128], BF16, tag="shiftT", name="shiftT")
        nc.vector.memset(shiftT[:], 0.0)
        nc.vector.tensor_copy(shiftT[0:64, 64:128], identb[0:64, 0:64])

        # ---------------- helpers ----------------
        def ln_tile(x_t, xn_t):
            """LayerNorm (no affine): xn = (x - mean) * rstd, one [128, D] tile."""
            xr = x_t.rearrange("p (s f) -> p s f", s=2)
            st = small.tile([128, 2, 6], F32, tag="ln_st", name="ln_st")
            nc.vector.bn_stats(st[:, 0, :], xr[:, 0, :])
            nc.vector.bn_stats(st[:, 1, :], xr[:, 1, :])
            mv = small.tile([128, 2], F32, tag="ln_mv", name="ln_mv")
            nc.vector.bn_aggr(mv[:], st[:])
            std = small.tile([128, 1], F32, tag="ln_std", name="ln_std")
            nc.scalar.activation(std[:], mv[:, 1:2], AF.Sqrt, bias=eps_t[:])
            rstd = small.tile([128, 1], F32, tag="ln_rstd", name="ln_rstd")
            nc.vector.reciprocal_approx_fast(rstd[:], std[:])
            nc.vector.tensor_scalar(xn_t, x_t, mv[:, 0:1], rstd[:],
                                    op0=ALU.subtract, op1=ALU.mult)

        def transpose_to(psum_pool, src, dsts, ident):
            for k, dst in enumerate(dsts):
                ps = psum_pool.tile([128, 128], src.dtype, tag="mm", name="tr")
                nc.tensor.transpose(ps[:], src[:, k * 128:(k + 1) * 128], ident)
                nc.any.tensor_copy(dst, ps[:])

        def make_xT(pool, psum_pool, x_src_tile_fn, ntiles, tagp,
                    fillers_at=None, out_list=None):
            """LN + transpose -> feature-major chunks [128, ntiles*128] bf16.
            fillers_at: {t: [closure, ...]} run after tile t's transposes."""
            xT = [pool.tile([128, ntiles * 128], BF16, tag=f"{tagp}_{k}",
                            name=f"{tagp}_{k}") for k in range(KD)]
            if out_list is not None:
                out_list.extend(xT)
            for t in range(ntiles):
                x_t = x_src_tile_fn(t)
                xn = xpool.tile([128, D], BF16, tag="xn", name="xn")
                ln_tile(x_t, xn[:])
                transpose_to(psum_pool, xn[:],
                             [xT[k][:, t * 128:(t + 1) * 128] for k in range(KD)],
                             identb[:])
                if fillers_at and t in fillers_at:
                    for f in fillers_at[t]:
                        f()
            return xT

        def bias_fm(bap, nslices, name):
            t = singles.tile([128, nslices], F32, tag=f"bfm_{name}", name=f"bfm_{name}")
            nc.sync.dma_start(t[:], bap.rearrange("(a p) -> p a", p=128))
            return t

        def bias_bc(pool, bap, width, name, dt=F32):
            t = pool.tile([128, width], dt, tag=f"bbc_{name}", name=f"bbc_{name}")
            eng = nc.sync if dt == F32 else nc.gpsimd
            eng.dma_start(t[:], bass.AP(tensor=bap.tensor, offset=bap.offset,
                                        ap=[[0, 128]] + bap.ap))
            return t

        def load_colblock(pool, w_ap, col0, ncols, tag, nk, bufs=2):
            t = pool.tile([128, nk, ncols], BF16, tag=tag, name=tag, bufs=bufs)
            src = w_ap[:, col0:col0 + ncols].rearrange("(a p) n -> p a n", p=128)
            nc.sync.dma_start(t[:], src)
            return t

        bq1_fm = bias_fm(bq1, KD, "bq1")
        bk1_fm = bias_fm(bk1, KD, "bk1")
        bq2_fm = bias_fm(bq2, KD, "bq2")
        bp_fm = bias_fm(bp, 2 * FF // 128, "bp")

        x1 = [xres.tile([128, D], F32, tag=f"x1_{t}", name=f"x1_{t}")
              for t in range(NT)]
        ctxT1 = [keep.tile([128, TC], BF16, tag=f"ctx1_{p}", name=f"ctx1_{p}")
                 for p in range(NP)]

        def xb_src(t):
            xt = xpool.tile([128, D], F32, tag="xt", name="xt")
            nc.sync.dma_start(xt[:], xb[t * 128:(t + 1) * 128, :])
            return xt[:]

        with tc.tile_pool(name="mm_ps", bufs=2, space="PSUM") as mm_ps, \
             tc.tile_pool(name="sc_ps", bufs=2, space="PSUM") as sc_ps, \
             tc.tile_pool(name="av_ps", bufs=1, space="PSUM") as av_ps, \
             tc.tile_pool(name="apool", bufs=2) as apool:

            def attn_pair(kTp, qTp, vslice_fn, ctx_dst, n_t, filler=(),
                          av_from_sc=False):
                """Both heads of a pair: scores -> exp -> (attn@V | denom),
                attn@V two key-tiles behind; `filler` closures are spread
                evenly into the PE-idle slots of the ACT-bound loop."""
                filler = list(filler)
                nsteps = (TC // 512) * (n_t + 2)
                due = [(len(filler) * (s + 1)) // nsteps for s in range(nsteps)]
                ndone = [0]
                step = [0]

                def fill():
                    want = due[min(step[0], nsteps - 1)]
                    while ndone[0] < want and ndone[0] < len(filler):
                        filler[ndone[0]]()
                        ndone[0] += 1
                    step[0] += 1

                LAG = 2

                def av_mm(avA, avB, pend_item, last):
                    pt, pexA, pexB = pend_item
                    nc.tensor.matmul(avA[0:65, :], vslice_fn(pt, 0), pexA[:],
                                     start=(pt == 0), stop=last)
                    nc.tensor.matmul(avB[0:65, :], vslice_fn(pt, 1), pexB[:],
                                     start=(pt == 0), stop=last)

                for j in range(TC // 512):
                    jsl = slice(j * 512, (j + 1) * 512)
                    if av_from_sc:
                        avA = sc_ps.tile([128, 512], F32, tag="scA", name="avA")
                        avB = sc_ps.tile([128, 512], F32, tag="scB", name="avB")
                    else:
                        avA = av_ps.tile([128, 512], F32, tag="avA", name="avA",
                                         bufs=1)
                        avB = av_ps.tile([128, 512], F32, tag="avB", name="avB",
                                         bufs=1)
                    pend = []
                    for t in range(n_t):
                        tsl = slice(t * 128, (t + 1) * 128)
                        scA = sc_ps.tile([128, 512], F32, tag="scA", name="scA")
                        scB = sc_ps.tile([128, 512], F32, tag="scB", name="scB")
                        nc.tensor.matmul(scA[:], kTp[0:64, tsl], qTp[0:64, jsl],
                                         start=True, stop=True)
                        nc.tensor.matmul(scB[:], kTp[64:128, tsl],
                                         qTp[64:128, jsl], start=True, stop=True)
                        exA = apool.tile([128, 512], BF16, tag="exA", name="exA",
                                         bufs=3)
                        exB = apool.tile([128, 512], BF16, tag="exB", name="exB",
                                         bufs=3)
                        nc.scalar.activation(exA[:], scA[:], AF.Exp)
                        nc.scalar.activation(exB[:], scB[:], AF.Exp)
                        pend.append((t, exA, exB))
                        if len(pend) > LAG:
                            av_mm(avA, avB, pend.pop(0), False)
                        fill()
                    while pend:
                        item = pend.pop(0)
                        av_mm(avA, avB, item, not pend)
                        fill()
                    dnA = small.tile([1, 512], F32, tag="dnA", name="dnA", bufs=1)
                    dnB = small.tile([1, 512], F32, tag="dnB", name="dnB", bufs=1)
                    nc.vector.tensor_copy(dnA[:], avA[64:65, :])
                    nc.vector.tensor_copy(dnB[:], avB[64:65, :])
                    rA = small.tile([1, 512], F32, tag="rA", name="rA", bufs=1)
                    rB = small.tile([1, 512], F32, tag="rB", name="rB", bufs=1)
                    nc.vector.reciprocal_approx_fast(rA[:], dnA[:])
                    nc.vector.reciprocal_approx_fast(rB[:], dnB[:])
                    rbA = apool.tile([64, 512], F32, tag="rbA", name="rbA", bufs=1)
                    rbB = apool.tile([64, 512], F32, tag="rbB", name="rbB", bufs=1)
                    nc.gpsimd.partition_broadcast(rbA[:], rA[:])
                    nc.gpsimd.partition_broadcast(rbB[:], rB[:])
                    fill()
                    nc.vector.tensor_mul(ctx_dst[0:64, jsl], avA[0:64, :], rbA[:])
                    cB = apool.tile([64, 512], BF16, tag="cB", name="cB", bufs=1)
                    nc.vector.tensor_mul(cB[:], avB[0:64, :], rbB[:])
                    ps2 = mm_ps.tile([128, 512], F32, tag="mm", name="shiftB")
                    nc.tensor.matmul(ps2[:], shiftT[:], cB[:], start=True,
                                     stop=True)
                    nc.any.tensor_copy(ctx_dst[64:128, jsl], ps2[64:128, :])
                    fill()
                while ndone[0] < len(filler):
                    filler[ndone[0]]()
                    ndone[0] += 1

            # =================================================================
            # Phase 1: self-attention
            # =================================================================
            p1stack = ExitStack()
            p1xnT = p1stack.enter_context(tc.tile_pool(name="p1xnT", bufs=1))
            p1w = p1stack.enter_context(tc.tile_pool(name="p1w", bufs=1))
            bv1_bc = bias_bc(p1xnT, bv1, D, "bv1", dt=BF16)

            xn1T = []  # filled by make_xT below; filler closures capture the list
            pair_state = [dict() for _ in range(NP)]
            vh_state = [dict() for _ in range(2)]

            def v_start(half):
                def go():
                    vh_state[half]['wvh'] = load_colblock(
                        p1w, wv1, half * 512, 512, "wvh", KD, bufs=1)
                    vh_state[half]['vsth'] = p1w.tile(
                        [128, NTB * 8 * 65], BF16, tag="vsth", name="vsth", bufs=2)
                return go

            def v_chunk(half, t):
                def go():
                    tsl = slice(t * 128, (t + 1) * 128)
                    ps = mm_ps.tile([128, 512], F32, tag="mm", name="vproj")
                    for k in range(KD):
                        nc.tensor.matmul(ps[:], xn1T[k][:, tsl],
                                         vh_state[half]['wvh'][:, k, :],
                                         start=(k == 0), stop=(k == KD - 1))
                    vt = vh_state[half]['vsth'][:, t * 520:(t + 1) * 520].rearrange(
                        "p (h c) -> p h c", h=8)
                    nc.vector.tensor_add(
                        vt[:, :, 0:64], ps[:].rearrange("p (h c) -> p h c", h=8),
                        bv1_bc[:, half * 512:half * 512 + 512].rearrange(
                            "p (h c) -> p h c", h=8))
                    nc.vector.memset(vt[:, :, 64:65], 1.0)
                return go

            def p_start(m):
                def go():
                    pair_state[m]['wkm'] = load_colblock(p1w, wk1, m * 128, 128,
                                                         "wkm", KD)
                    pair_state[m]['wqm'] = load_colblock(p1w, wq1, m * 128, 128,
                                                         "wqm", KD)
                    pair_state[m]['kT'] = p1w.tile([128, N], BF16, tag="kT",
                                                   name="kT", bufs=2)
                    pair_state[m]['qT'] = p1w.tile([128, TC], BF16, tag="qT",
                                                   name="qT", bufs=2)
                return go

            def k_chunk(m, jb):
                def go():
                    jbs = slice(jb * 512, (jb + 1) * 512)
                    ps = mm_ps.tile([128, 512], F32, tag="mm", name="kproj")
                    for k in range(KD):
                        nc.tensor.matmul(ps[:], pair_state[m]['wkm'][:, k, :],
                                         xn1T[k][:, jbs], start=(k == 0),
                                         stop=(k == KD - 1))
                    nc.vector.tensor_scalar(pair_state[m]['kT'][:, jbs], ps[:],
                                            bk1_fm[:, m:m + 1], None, op0=ALU.add)
                return go

            def q_chunk(m, jb):
                def go():
                    jbs = slice(jb * 512, (jb + 1) * 512)
                    ps = mm_ps.tile([128, 512], F32, tag="mm", name="qproj")
                    for k in range(KD):
                        nc.tensor.matmul(ps[:], pair_state[m]['wqm'][:, k, :],
                                         xn1T[k][:, jbs], start=(k == 0),
                                         stop=(k == KD - 1))
                    nc.vector.tensor_scalar(pair_state[m]['qT'][:, jbs], ps[:],
                                            bq1_fm[:, m:m + 1], None, op0=ALU.add)
                return go

            def pair_chunks(m):
                return ([p_start(m)] + [k_chunk(m, jb) for jb in range(4)]
                        + [q_chunk(m, jb) for jb in range(2)])

            # ---- P2 K/V prep chunks (run during pair 7's attention) ----
            ctx_sb = keep.tile([128, CD], F32, tag="ctx_sb", name="ctx_sb")
            cT = [keep.tile([128, 128], BF16, tag=f"cT_{k}", name=f"cT_{k}")
                  for k in range(KC)]
            k2p = [keep.tile([128, 128], BF16, tag=f"k2p_{m}", name=f"k2p_{m}")
                   for m in range(NP)]
            v2st = keep.tile([128, H * 65], BF16, tag="v2st", name="v2st")
            p2k_state = {}

            def p2_prep_chunks():
                out = []

                def c_load():
                    nc.sync.dma_start(ctx_sb[:], ctxp)
                    transpose_to(mm_ps, ctx_sb[:], [cT[k][:] for k in range(KC)],
                                 identf[:])
                out.append(c_load)

                def k2_chunk(m2):
                    def go():
                        wkm2 = load_colblock(p1w, wk2, m2 * 128, 128, "wk2m", KC, bufs=1)
                        ps = mm_ps.tile([128, 512], F32, tag="mm", name="k2proj")
                        for k in range(KC):
                            nc.tensor.matmul(ps[:, 0:128], wkm2[:, k, :],
                                             cT[k][:], start=(k == 0),
                                             stop=(k == KC - 1))
                        nc.any.tensor_copy(k2p[m2][:], ps[:, 0:128])
                    return go
                out.extend(k2_chunk(m2) for m2 in range(NP))

                def v2_chunk(n):
                    def go():
                        wvn = load_colblock(p1w, wv2, n * 512, 512, "wv2n", KC,
                                            bufs=1)
                        ps = mm_ps.tile([128, 512], F32, tag="mm", name="v2proj")
                        for k in range(KC):
                            nc.tensor.matmul(ps[:], cT[k][:], wvn[:, k, :],
                                             start=(k == 0), stop=(k == KC - 1))
                        dst = v2st[:].rearrange("p (h c) -> p h c", h=H)[
                            :, n * 8:(n + 1) * 8, 0:64]
                        nc.vector.tensor_copy(
                            dst, ps[:].rearrange("p (h c) -> p h c", h=8))
                    return go
                out.extend(v2_chunk(n) for n in range(2))

                def ones_load():
                    onescol = v2st[:].rearrange("p (h c) -> p h c", h=H)[:, :, 64:65]
                    nc.sync.dma_start(onescol,
                                      vones.rearrange("p (h o) -> p h o", o=1))
                out.append(ones_load)
                return out

            # ---- make_xT with V-proj + pair-0 fillers ----
            fillers_at = {0: [v_start(0)]}
            for t in range(NTB):
                fillers_at.setdefault(t, []).append(v_chunk(0, t))
            fillers_at[1].append(p_start(0))
            for jb in range(4):
                fillers_at.setdefault(4 * jb + 3, []).append(k_chunk(0, jb))
            for jb in range(2):
                fillers_at.setdefault(8 * jb + 7, []).append(q_chunk(0, jb))
            make_xT(p1xnT, mm_ps, xb_src, NTB, "xn1T", fillers_at,
                    out_list=xn1T)

            # ---- attention over pairs with interleaved fillers ----
            vh1 = [v_start(1)] + [v_chunk(1, t) for t in range(NTB)]
            fill_for = {
                0: pair_chunks(1),
                1: pair_chunks(2) + vh1[0:6],
                2: pair_chunks(3) + vh1[6:12],
                3: vh1[12:17] + pair_chunks(4),
                4: pair_chunks(5),
                5: pair_chunks(6),
                6: pair_chunks(7),
                7: p2_prep_chunks(),
            }
            for m in range(NP):
                half, pi = m // 4, m % 4
                attn_pair(pair_state[m]['kT'][:], pair_state[m]['qT'][:],
                          lambda t, hh, half=half, pi=pi: vh_state[half]['vsth'][
                              :, t * 520 + (pi * 2 + hh) * 65:
                              t * 520 + (pi * 2 + hh + 1) * 65],
                          ctxT1[m][:], NTB, fill_for[m])

            p1stack.close()

            # ---- O1 projection + residual ----
            with tc.tile_pool(name="p1o", bufs=1) as p1o:
                bo1_bc = bias_bc(p1o, bo1, D, "bo1")
                wo1p = [p1o.tile([128, D], BF16, tag=f"wo1_{p}", name=f"wo1_{p}")
                        for p in range(NP)]
                for p in range(NP):
                    nc.sync.dma_start(wo1p[p][:], wo1[p * 128:(p + 1) * 128, :])
                for t in range(NT):
                    tsl = slice(t * 128, (t + 1) * 128)
                    xo = xpool.tile([128, D], F32, tag="xt", name="xt_res")
                    nc.sync.dma_start(xo[:], xb[t * 128:(t + 1) * 128, :])
                    ps0 = mm_ps.tile([128, 512], F32, tag="mm", name="oproj0")
                    ps1 = mm_ps.tile([128, 512], F32, tag="mm", name="oproj1")
                    for p in range(NP):
                        nc.tensor.matmul(ps0[:], ctxT1[p][:, tsl],
                                         wo1p[p][:, 0:512], start=(p == 0),
                                         stop=(p == NP - 1))
                        nc.tensor.matmul(ps1[:], ctxT1[p][:, tsl],
                                         wo1p[p][:, 512:1024], start=(p == 0),
                                         stop=(p == NP - 1))
                    for n, ps in ((0, ps0), (1, ps1)):
                        nsl = slice(n * 512, (n + 1) * 512)
                        tmp = evpool.tile([128, 512], F32, tag="otmp", name="otmp")
                        nc.any.tensor_add(tmp[:], ps[:], bo1_bc[:, nsl])
                        nc.vector.tensor_add(x1[t][:, nsl], tmp[:], xo[:, nsl])

            # =================================================================
            # Phase 2: cross-attention (updates x1 in place)
            # =================================================================
            with tc.tile_pool(name="p2", bufs=1) as p2, \
                 tc.tile_pool(name="p2w", bufs=2) as p2w:
                ctxT2 = [p2.tile([128, TC], BF16, tag=f"ctx2_{p}", name=f"ctx2_{p}")
                         for p in range(NP)]
                xn2T = []
                q2_state = [dict() for _ in range(NP)]

                def q2_start(m):
                    def go():
                        q2_state[m]['wqm'] = load_colblock(p2w, wq2, m * 128, 128,
                                                           f"wq2m_{m}", KD, bufs=1)
                        q2_state[m]['qT'] = p2w.tile([128, TC], BF16,
                                                     tag=f"q2T_{m}",
                                                     name="q2T", bufs=1)
                    return go

                def q2_chunk(m, jb):
                    def go():
                        jbs = slice(jb * 512, (jb + 1) * 512)
                        ps = mm_ps.tile([128, 512], F32, tag="mm", name="q2proj")
                        for k in range(KD):
                            nc.tensor.matmul(ps[:], q2_state[m]['wqm'][:, k, :],
                                             xn2T[k][:, jbs], start=(k == 0),
                                             stop=(k == KD - 1))
                        nc.vector.tensor_scalar(q2_state[m]['qT'][:, jbs], ps[:],
                                                bq2_fm[:, m:m + 1], None,
                                                op0=ALU.add)
                    return go

                def q2_chunks(m):
                    return [q2_start(m), q2_chunk(m, 0), q2_chunk(m, 1)]

                f2 = {0: [q2_start(0)], 3: [q2_chunk(0, 0)], 7: [q2_chunk(0, 1)]}
                make_xT(p2, mm_ps, lambda t: x1[t][:], NT, "xn2T", f2,
                        out_list=xn2T)
                for m in range(NP):
                    attn_pair(k2p[m][:], q2_state[m]['qT'][:],
                              lambda t, hh, m=m: v2st[:, (2 * m + hh) * 65:
                                                      (2 * m + hh + 1) * 65],
                              ctxT2[m][:], 1,
                              q2_chunks(m + 1) if m + 1 < NP else ())

                with tc.tile_pool(name="p2o", bufs=1) as p2o:
                    bo2_bc = bias_bc(p2o, bo2, D, "bo2")
                    wo2p = [p2o.tile([128, D], BF16, tag=f"wo2_{p}", name=f"wo2_{p}")
                            for p in range(NP)]
                    for p in range(NP):
                        nc.sync.dma_start(wo2p[p][:], wo2[p * 128:(p + 1) * 128, :])
                    for t in range(NT):
                        tsl = slice(t * 128, (t + 1) * 128)
                        ps0 = mm_ps.tile([128, 512], F32, tag="mm", name="o2proj0")
                        ps1 = mm_ps.tile([128, 512], F32, tag="mm", name="o2proj1")
                        for p in range(NP):
                            nc.tensor.matmul(ps0[:], ctxT2[p][:, tsl],
                                             wo2p[p][:, 0:512], start=(p == 0),
                                             stop=(p == NP - 1))
                            nc.tensor.matmul(ps1[:], ctxT2[p][:, tsl],
                                             wo2p[p][:, 512:1024], start=(p == 0),
                                             stop=(p == NP - 1))
                        for n, ps in ((0, ps0), (1, ps1)):
                            nsl = slice(n * 512, (n + 1) * 512)
                            tmp = evpool.tile([128, 512], F32, tag="otmp",
                                              name="o2tmp")
                            nc.any.tensor_add(tmp[:], ps[:], bo2_bc[:, nsl])
                            nc.vector.tensor_add(x1[t][:, nsl], tmp[:],
                                                 x1[t][:, nsl])

        # =================================================================
        # Phase 3: GeGLU FFN (x1 now holds x2); FF2 partials DMA-accumulated
        # =================================================================
        with tc.tile_pool(name="mm3_ps", bufs=4, space="PSUM") as mm3, \
             tc.tile_pool(name="p3", bufs=1) as p3:
            xn3T = make_xT(p3, mm3, lambda t: x1[t][:], NT, "xn3T")
            bf_bc = bias_bc(p3, bf, D, "bf")
            # x1 += bf (safe: xn3T already computed)
            for t in range(NT):
                nc.vector.tensor_add(x1[t][:], x1[t][:], bf_bc[:])

            NG = 2
            MPG = FF // 128 // NG
            with tc.tile_pool(name="p3w", bufs=1) as p3w:
                for G in range(NG):
                    wfn = [p3w.tile([128, D], BF16, tag=f"wf_{i}", name=f"wf_{i}",
                                    bufs=1) for i in range(MPG)]
                    for i in range(MPG):
                        nc.sync.dma_start(wfn[i][:],
                                          wf[(G * MPG + i) * 128:
                                             (G * MPG + i + 1) * 128, :])
                    agT = [p3w.tile([128, TC], BF16, tag=f"agT_{i}",
                                    name=f"agT_{i}", bufs=1) for i in range(MPG)]
                    for i in range(MPG):
                        mc = G * MPG + i
                        wpa = load_colblock(p3w, wp, mc * 128, 128, "wpa", KD)
                        wpg = load_colblock(p3w, wp, FF + mc * 128, 128, "wpg", KD)
                        for jb in range(TC // 512):
                            jsl = slice(jb * 512, (jb + 1) * 512)
                            ps_a = mm3.tile([128, 512], F32, tag="mm", name="ff1a")
                            for k in range(KD):
                                nc.tensor.matmul(ps_a[:], wpa[:, k, :],
                                                 xn3T[k][:, jsl], start=(k == 0),
                                                 stop=(k == KD - 1))
                            ps_g = mm3.tile([128, 512], F32, tag="mm", name="ff1g")
                            for k in range(KD):
                                nc.tensor.matmul(ps_g[:], wpg[:, k, :],
                                                 xn3T[k][:, jsl], start=(k == 0),
                                                 stop=(k == KD - 1))
                            gel = evpool.tile([128, 512], F32, tag="gel",
                                              name="gel")
                            nc.scalar.activation(gel[:], ps_g[:], AF.Gelu,
                                                 bias=bp_fm[:, FF // 128 + mc:
                                                            FF // 128 + mc + 1])
                            nc.vector.scalar_tensor_tensor(agT[i][:, jsl], ps_a[:],
                                                           bp_fm[:, mc:mc + 1],
                                                           gel[:], op0=ALU.add,
                                                           op1=ALU.mult)
                    for t in range(NT):
                        tsl = slice(t * 128, (t + 1) * 128)
                        ps0 = mm3.tile([128, 512], F32, tag="mm", name="ff2a")
                        ps1 = mm3.tile([128, 512], F32, tag="mm", name="ff2b")
                        for i in range(MPG):
                            nc.tensor.matmul(ps0[:], agT[i][:, tsl],
                                             wfn[i][:, 0:512], start=(i == 0),
                                             stop=(i == MPG - 1))
                            nc.tensor.matmul(ps1[:], agT[i][:, tsl],
                                             wfn[i][:, 512:1024], start=(i == 0),
                                             stop=(i == MPG - 1))
                        for n, ps in ((0, ps0), (1, ps1)):
                            nsl = slice(n * 512, (n + 1) * 512)
                            ev = evpool.tile([128, 512], F32, tag="yev", name="yev",
                                             bufs=3)
                            if G == 0:
                                nc.vector.tensor_add(ev[:], ps[:], x1[t][:, nsl])
                                nc.sync.dma_start(y[t * 128:(t + 1) * 128, nsl],
                                                  ev[:])
                            else:
                                nc.any.tensor_copy(ev[:], ps[:])
                                nc.gpsimd.dma_start(y[t * 128:(t + 1) * 128, nsl],
                                                    ev[:], accum_op=ALU.add)

    nc.compile()
    return nc


def _prep_inputs(inputs):
    """Host-side weight transforms + per-core input maps."""
    f = np.float32
    bff = ml_dtypes.bfloat16
    x = np.asarray(inputs["x"], f)
    context = np.asarray(inputs["context"], f)
    g1, b1 = np.asarray(inputs["g1"], f), np.asarray(inputs["b1"], f)
    g2, b2 = np.asarray(inputs["g2"], f), np.asarray(inputs["b2"], f)
    g3, b3 = np.asarray(inputs["g3"], f), np.asarray(inputs["b3"], f)
    sc = f(DH ** -0.5)

    Wq1, Wk1, Wv1 = (np.asarray(inputs[k], f) for k in ("Wq1", "Wk1", "Wv1"))
    Wq2, Wp = np.asarray(inputs["Wq2"], f), np.asarray(inputs["Wp"], f)

    def c(a, dt=None):
        a = np.ascontiguousarray(a)
        return a.astype(dt) if dt is not None else a

    shared = dict(
        wq1=c(g1[:, None] * Wq1 * sc, bff), bq1=c(b1 @ Wq1 * sc),
        wk1=c(g1[:, None] * Wk1, bff), bk1=c(b1 @ Wk1),
        wv1=c(g1[:, None] * Wv1, bff), bv1=c(b1 @ Wv1),
        wo1=c(np.asarray(inputs["Wo1"], f), bff), bo1=c(np.asarray(inputs["bo1"], f)),
        wq2=c(g2[:, None] * Wq2 * sc, bff), bq2=c(b2 @ Wq2 * sc),
        wk2=c(np.asarray(inputs["Wk2"], f), bff),
        wv2=c(np.asarray(inputs["Wv2"], f), bff),
        wo2=c(np.asarray(inputs["Wo2"], f), bff), bo2=c(np.asarray(inputs["bo2"], f)),
        wp=c(g3[:, None] * Wp, bff),
        bp=c(np.asarray(inputs["bp"], f) + b3 @ Wp),
        wf=c(np.asarray(inputs["Wf"], f), bff), bf=c(np.asarray(inputs["bf"], f)),
        vones=np.ascontiguousarray(np.where(np.arange(128)[:, None] < S, 1, 0).repeat(H, 1).astype(bff)),
    )
    ctxpad = np.zeros((B, 128, CD), f)
    ctxpad[:, :S, :] = context

    in_maps = []
    for cid in range(NCORES):
        b, half = cid // 2, cid % 2
        m = dict(shared)
        m["xb"] = np.ascontiguousarray(np.roll(x[b], -half * TC, axis=0))
        m["ctxp"] = np.ascontiguousarray(ctxpad[b])
        in_maps.append(m)
    return in_maps


def run(inputs, trace=False):
    if "nc" not in _CACHE:
        _CACHE["nc"] = build_program()
    nc = _CACHE["nc"]
    in_maps = _prep_inputs(inputs)
    res = bass_utils.run_bass_kernel_spmd(nc, in_maps, core_ids=list(range(NCORES)),
                                          trace=trace)
    out = np.empty((B, N, D), np.float32)
    for cid in range(NCORES):
        b, half = cid // 2, cid % 2
        out[b, half * TC:(half + 1) * TC] = res.results[cid]["y"]
    return out, res


def kernel(**inputs):
    out, _ = run(inputs, trace=False)
    return out
